# revision 1
# baseline (speedup 1.0000x reference)
"""Trainium2 Bass kernel for nn_Encoder_4724464025749 (tree-GRU encoder).

Strategy
--------
Pure data parallelism: batch B=4096 is split across 8 NeuronCores (512
columns each).  Each core runs the full 127-node binary-tree recursion for
its batch shard with all tensors kept feature-major ([feature partitions,
batch columns]) so every matmul contracts over the partition dimension and
hidden states never leave SBUF.  Per core the 512 columns are further split
into SPLITS independent tree walks, emitted interleaved in post-order, so
the Tile scheduler always has several independent nodes in flight (keeps
the PE dense and at full clock).

Precision: the attention normalization a = s / (s0 + s1) makes the model
chaotic — matmul noise of 1e-4 (f32r) or 4e-3 (bf16) explodes to 20%+ output
error, so hidden-state matmuls run in true fp32.  The x projections use an
exact trick: operands are pre-split on the host into 11-bit hi/lo halves so
a single-pass f32r matmul over a stacked K computes W@x to fp32 accuracy at
1 cycle/row (f32r rounds operands to 11 mantissa bits; pre-rounded operands
pass through exactly).
"""

import numpy as np

DEPTH = 7
H = 256
I = 32
O = 128
B = 4096
NCORES = 8
P = 128
HT = H // P          # feature tiles per vector
KSP = 3 * I + 2      # split x contraction: xhi | 1 | xlo | 1 | xhi
CH = 4               # nodes per x/mask DMA chunk
NCOL = B // NCORES   # batch columns per core
SPLITS = 2           # independent tree walks per core


def _post_order(depth, block=4):
    """Post-order walk, but subtrees rooted at `block` level are emitted
    internally in bottom-up level order (wider ready-set for the scheduler
    while keeping the DFS-bounded live set above the block level)."""
    order = []

    def rec(d, j):
        if d == block and depth - 1 > d:
            for dd in range(depth - 1, d - 1, -1):
                for jj in range(j << (dd - d), (j + 1) << (dd - d)):
                    order.append((dd, jj))
            return
        if d < depth - 1:
            rec(d + 1, 2 * j)
            rec(d + 1, 2 * j + 1)
        order.append((d, j))

    rec(0, 0)
    return order


def _gid(d, j):
    return 2 ** d - 1 + j


def _round11(x):
    """Round fp32 to 11 explicit mantissa bits (the f32r operand grid)."""
    x = np.ascontiguousarray(np.asarray(x, dtype=np.float32))
    b = x.view(np.uint32)
    r = ((b + np.uint32(0x800)) >> np.uint32(12)) << np.uint32(12)
    return r.view(np.float32)


_MODULE_CACHE = {}


def _build_module(depth=DEPTH, ncol=NCOL, use_bias=False, mode="f32",
                  num_devices=NCORES, splits=SPLITS, use_mask=True):
    key = (depth, ncol, use_bias, mode, num_devices, splits, use_mask)
    if key in _MODULE_CACHE:
        return _MODULE_CACHE[key]

    import concourse.mybir as mybir
    import concourse.tile as tile
    from concourse import bacc

    dt = mybir.dt
    ACT_F = mybir.ActivationFunctionType
    ALU = mybir.AluOpType
    adt = {"f32": dt.float32, "f32r": dt.float32r,
           "bf16": dt.bfloat16}[mode]            # storage + H-matmul dtype
    xdt = dt.bfloat16 if mode == "bf16" else dt.float32r  # split-x path

    nodes = 2 ** depth - 1
    order = _post_order(depth)
    nsub = ncol // splits                        # columns per tree walk

    nc = bacc.Bacc("TRN2", num_devices=num_devices, debug=False)

    xT_d = nc.dram_tensor("xT", [KSP, nodes, ncol], xdt, kind="ExternalInput").ap()
    mb_d = nc.dram_tensor("maskb", [P, nodes, ncol], adt, kind="ExternalInput").ap()
    wi_d = nc.dram_tensor("wi", [KSP, 3 * H], xdt, kind="ExternalInput").ap()
    whr_d = nc.dram_tensor("whr", [P, HT, H], adt, kind="ExternalInput").ap()
    whz_d = nc.dram_tensor("whz", [P, HT, H], adt, kind="ExternalInput").ap()
    whn_d = nc.dram_tensor("whn", [P, HT, H], adt, kind="ExternalInput").ap()
    wa_d = nc.dram_tensor("wa", [P, HT, H], adt, kind="ExternalInput").ap()
    wms0_d = nc.dram_tensor("wms0", [P, HT, H], adt, kind="ExternalInput").ap()
    wms1_d = nc.dram_tensor("wms1", [P, HT, H], adt, kind="ExternalInput").ap()
    wsc_d = nc.dram_tensor("wsc", [P, HT, 1], adt, kind="ExternalInput").ap()
    wout_d = nc.dram_tensor("wout", [P, HT, 2 * O], adt, kind="ExternalInput").ap()
    ones_d = nc.dram_tensor("ones1", [1, P], adt, kind="ExternalInput").ap()
    bias_d = nc.dram_tensor("biases", [P, 9], dt.float32, kind="ExternalInput").ap()
    out_d = nc.dram_tensor("out", [2, P, ncol], dt.float32,
                           kind="ExternalOutput").ap()

    with tile.TileContext(nc) as tc:
        with tc.tile_pool(name="wpool", bufs=1) as wpool, \
             tc.tile_pool(name="xpool", bufs=2 * splits) as xpool, \
             tc.tile_pool(name="mpool", bufs=2 * splits) as mpool, \
             tc.tile_pool(name="hpool", bufs=13 * splits) as hpool, \
             tc.tile_pool(name="vpool", bufs=17 * splits) as vpool, \
             tc.tile_pool(name="spool", bufs=3 * splits) as spool, \
             tc.tile_pool(name="opool", bufs=2) as opool, \
             tc.tile_pool(name="ppool", bufs=8, space="PSUM") as ppool:

            # ---- load weights once ----
            def wtile(dram, shape, dtype):
                t = wpool.tile(shape, dtype, tag=dram.name, name="w_" + dram.name)
                nc.sync.dma_start(out=t[:], in_=dram[:])
                return t

            wi_t = wtile(wi_d, [KSP, 3 * H], xdt)
            whr_t = wtile(whr_d, [P, HT, H], adt)
            whz_t = wtile(whz_d, [P, HT, H], adt)
            whn_t = wtile(whn_d, [P, HT, H], adt)
            wa_t = wtile(wa_d, [P, HT, H], adt)
            wms_t = [wtile(wms0_d, [P, HT, H], adt), wtile(wms1_d, [P, HT, H], adt)]
            wsc_t = wtile(wsc_d, [P, HT, 1], adt)
            wout_t = wtile(wout_d, [P, HT, 2 * O], adt)
            ones_t = wtile(ones_d, [1, P], adt)
            bias_t = wpool.tile([P, 9], dt.float32, tag="biases", name="biases_t")
            nc.sync.dma_start(out=bias_t[:], in_=bias_d[:])

            # chunked x / mask staging, per tree walk
            x_tiles = {}
            m_tiles = {}

            def get_chunk(w, t):
                c = t // CH
                if (w, c) not in x_tiles:
                    n0 = c * CH
                    n1 = min(n0 + CH, nodes)
                    c0, c1 = w * nsub, (w + 1) * nsub
                    xt = xpool.tile([KSP, CH, nsub], xdt, tag="xchunk",
                                    name="xchunk")
                    nc.sync.dma_start(out=xt[:, : n1 - n0, :],
                                      in_=xT_d[:, n0:n1, c0:c1])
                    if use_mask:
                        mt = mpool.tile([P, CH, nsub], adt, tag="mchunk",
                                        name="mchunk")
                        nc.sync.dma_start(out=mt[:, : n1 - n0, :],
                                          in_=mb_d[:, n0:n1, c0:c1])
                    else:
                        mt = None
                    x_tiles[(w, c)] = xt
                    m_tiles[(w, c)] = mt
                return x_tiles[(w, c)], m_tiles[(w, c)], t - c * CH

            def psum_tile():
                return ppool.tile([P, HT, nsub], dt.float32, tag="ps", name="ps")

            def work_tile():
                return vpool.tile([P, HT, nsub], adt, tag="work", name="work")

            def mm_h(ps, w_t, rhs, mt, start, stop):
                """ps[:, mt] (+)= w_t[:, :, mt*P:(mt+1)*P].T @ rhs  (K=H)."""
                for kt in range(HT):
                    nc.tensor.matmul(
                        ps[:, mt, :],
                        lhsT=w_t[:, kt, mt * P:(mt + 1) * P],
                        rhs=rhs[:, kt, :],
                        start=(start and kt == 0),
                        stop=(stop and kt == HT - 1),
                    )

            def mm_x(ps, mt, col0, xc, xi, start, stop):
                """ps[:, mt] (+)= wi[:, col0+mt*P : col0+(mt+1)*P].T @ x."""
                nc.tensor.matmul(
                    ps[:, mt, :],
                    lhsT=wi_t[:, col0 + mt * P: col0 + (mt + 1) * P],
                    rhs=xc[:, xi, :],
                    start=start,
                    stop=stop,
                )

            def act(out_ap, in_ap, func, bias=0.0):
                nc.scalar.activation(out_ap, in_ap, func, bias=bias)

            def emit_leaf(w, t):
                xc, mc, xi = get_chunk(w, t)
                # z = sigmoid(Wiz x + bz) ; n = tanh(Win x + bn)
                psz = psum_tile()
                for mt in range(HT):
                    mm_x(psz, mt, H, xc, xi, True, True)
                z = work_tile()
                act(z[:], psz[:], ACT_F.Sigmoid)
                psn = psum_tile()
                for mt in range(HT):
                    mm_x(psn, mt, 2 * H, xc, xi, True, True)
                n = work_tile()
                act(n[:], psn[:], ACT_F.Tanh)
                # h = (1-z)*n * m = (n - z*n) * m
                t1 = work_tile()
                nc.vector.tensor_mul(t1[:], z[:], n[:])
                h = hpool.tile([P, HT, nsub], adt, tag="h", name="h")
                if use_mask:
                    nc.vector.tensor_sub(t1[:], n[:], t1[:])
                    mbc = mc[:, xi:xi + 1, :].to_broadcast((P, HT, nsub))
                    nc.vector.tensor_mul(h[:], t1[:], mbc)
                else:
                    nc.vector.tensor_sub(h[:], n[:], t1[:])
                return h

            def emit_internal(w, t, d, hl, hr):
                xc, mc, xi = get_chunk(w, t)
                kids = (hl, hr)

                # ---- r_k = sigmoid(xi_r + Whr c_k + b_r) ; s = sum r_k*c_k ----
                r = []
                for k in range(2):
                    psr = psum_tile()
                    for mt in range(HT):
                        mm_x(psr, mt, 0, xc, xi, True, False)
                        mm_h(psr, whr_t, kids[k], mt, False, True)
                    rk = work_tile()
                    act(rk[:], psr[:], ACT_F.Sigmoid)
                    r.append(rk)
                s = work_tile()
                nc.vector.tensor_mul(s[:], r[0][:], hl[:])
                t3 = work_tile()
                nc.vector.tensor_mul(t3[:], r[1][:], hr[:])
                nc.vector.tensor_add(s[:], s[:], t3[:])

                # ---- attention: ms_k = tanh(Wms_k c_k + b_k) ----
                ms = []
                for k in range(2):
                    psm = psum_tile()
                    for mt in range(HT):
                        mm_h(psm, wms_t[k], kids[k], mt, True, True)
                    mk = work_tile()
                    if use_bias:
                        for mt in range(HT):
                            act(mk[:, mt, :], psm[:, mt, :], ACT_F.Tanh,
                                bias=bias_t[:, 2 * k + mt: 2 * k + mt + 1])
                    else:
                        act(mk[:], psm[:], ACT_F.Tanh)
                    ms.append(mk)

                # ---- scores s_k = w . ms_k (+ w_b) ----
                # child-k score goes to partition 0, free range k, of one
                # psum bank so all downstream row ops share base partition 0
                pss = psum_tile()
                for k in range(2):
                    for kt in range(HT):
                        nc.tensor.matmul(
                            pss[0:1, k, :],
                            lhsT=wsc_t[:, kt, :],
                            rhs=ms[k][:, kt, :],
                            start=(kt == 0),
                            stop=(kt == HT - 1),
                        )
                sc = spool.tile([1, 2, nsub], dt.float32, tag="sc", name="sc")
                nc.vector.tensor_copy(sc[:], pss[0:1, :, :])
                if use_bias:
                    nc.vector.tensor_scalar(sc[:], sc[:],
                                            bias_t[0:1, 8:9], None, ALU.add)
                s0, s1 = sc[:, 0, :], sc[:, 1, :]
                den = spool.tile([1, nsub], dt.float32, tag="den", name="den")
                nc.vector.tensor_add(den[:], s0, s1)
                rec = spool.tile([1, nsub], dt.float32, tag="rec", name="rec")
                nc.vector.reciprocal_approx_fast(rec[:], den[:])
                a0 = spool.tile([1, nsub], adt, tag="a01", name="a0")
                a1 = spool.tile([1, nsub], adt, tag="a01", name="a1")
                nc.vector.tensor_mul(a0[:], s0, rec[:])
                nc.vector.tensor_mul(a1[:], s1, rec[:])

                # broadcast a0/a1 across partitions via K=1 ones matmul
                psb = psum_tile()
                for k, ak in enumerate((a0, a1)):
                    nc.tensor.matmul(psb[:, k, :], lhsT=ones_t[:],
                                     rhs=ak[:], start=True, stop=True)
                # ---- g = a0*c0 + a1*c1 ; cs = tanh(Wa g + wa_b) ----
                g = work_tile()
                nc.vector.tensor_mul(
                    g[:], hl[:], psb[:, 0:1, :].to_broadcast((P, HT, nsub)))
                t1 = work_tile()
                nc.vector.tensor_mul(
                    t1[:], hr[:], psb[:, 1:2, :].to_broadcast((P, HT, nsub)))
                nc.vector.tensor_add(g[:], g[:], t1[:])
                psc = psum_tile()
                for mt in range(HT):
                    mm_h(psc, wa_t, g, mt, True, True)
                cs = work_tile()
                if use_bias:
                    for mt in range(HT):
                        act(cs[:, mt, :], psc[:, mt, :], ACT_F.Tanh,
                            bias=bias_t[:, 4 + mt: 5 + mt])
                else:
                    act(cs[:], psc[:], ACT_F.Tanh)

                # ---- z, n, h ----
                psz = psum_tile()
                for mt in range(HT):
                    mm_x(psz, mt, H, xc, xi, True, False)
                    mm_h(psz, whz_t, cs, mt, False, True)
                z = work_tile()
                act(z[:], psz[:], ACT_F.Sigmoid)
                psn = psum_tile()
                for mt in range(HT):
                    mm_x(psn, mt, 2 * H, xc, xi, True, False)
                    mm_h(psn, whn_t, s, mt, False, True)
                n = work_tile()
                act(n[:], psn[:], ACT_F.Tanh)
                # h = (n + z*(cs - n)) * m
                t4 = work_tile()
                nc.vector.tensor_sub(t4[:], cs[:], n[:])
                nc.vector.tensor_mul(t4[:], z[:], t4[:])
                h = hpool.tile([P, HT, nsub], adt, tag="h", name="h")
                if use_mask:
                    nc.vector.tensor_add(t4[:], n[:], t4[:])
                    mbc = mc[:, xi:xi + 1, :].to_broadcast((P, HT, nsub))
                    nc.vector.tensor_mul(h[:], t4[:], mbc)
                else:
                    nc.vector.tensor_add(h[:], n[:], t4[:])
                return h

            # ---- walk the trees in interleaved post-order ----
            # stagger the walks so their PE-heavy and DVE/ACT-heavy phases
            # anti-align instead of bursting together
            LAG = 0
            hmaps = [{} for _ in range(splits)]
            roots = [None] * splits

            def emit_one(w, t):
                d, j = order[t]
                hmap = hmaps[w]
                if d == depth - 1:
                    hmap[(d, j)] = emit_leaf(w, t)
                else:
                    hl = hmap.pop((d + 1, 2 * j))
                    hr = hmap.pop((d + 1, 2 * j + 1))
                    hmap[(d, j)] = emit_internal(w, t, d, hl, hr)

            n_nodes = len(order)
            for t in range(n_nodes + LAG * (splits - 1)):
                for w in range(splits):
                    tw = t - LAG * w
                    if 0 <= tw < n_nodes:
                        emit_one(w, tw)
            for w in range(splits):
                roots[w] = hmaps[w][(0, 0)]

            # ---- output heads ----
            for w in range(splits):
                root = roots[w]
                c0, c1 = w * nsub, (w + 1) * nsub
                pso = psum_tile()
                for oi in range(2):
                    for kt in range(HT):
                        nc.tensor.matmul(
                            pso[:, oi, :],
                            lhsT=wout_t[:, kt, oi * O:(oi + 1) * O],
                            rhs=root[:, kt, :],
                            start=(kt == 0),
                            stop=(kt == HT - 1),
                        )
                for oi in range(2):
                    ot = opool.tile([P, nsub], dt.float32, tag="osb", name="osb")
                    if use_bias:
                        act(ot[:], pso[:, oi, :], ACT_F.Identity,
                            bias=bias_t[:, 6 + oi: 7 + oi])
                    else:
                        act(ot[:], pso[:, oi, :], ACT_F.Identity)
                    nc.sync.dma_start(out=out_d[oi, :, c0:c1], in_=ot[:])

    nc.compile()
    _MODULE_CACHE[key] = nc
    return nc


def _to_dt(arr, mode):
    if mode == "bf16":
        import ml_dtypes
        return np.asarray(arr, dtype=np.float32).astype(ml_dtypes.bfloat16)
    return np.ascontiguousarray(np.asarray(arr, dtype=np.float32))


def _pack_weights(inputs, mode):
    """Host-side packing of weights into device lhsT layouts."""
    f32 = lambda k: np.asarray(inputs[k], dtype=np.float32)

    def lhsT_h(w):  # [H, H] torch-layout -> [P, HT, H]
        return w.T.reshape(HT, P, w.shape[0]).transpose(1, 0, 2)

    wir_w, wiz_w, win_w = f32("wir_w"), f32("wiz_w"), f32("win_w")
    br = f32("wir_b") + f32("whr_b")
    bz = f32("wiz_b") + f32("whz_b")
    bn = f32("win_b") + f32("whn_b")
    wcat = np.concatenate([wir_w, wiz_w, win_w], axis=0)      # [3H, I]
    bcat = np.concatenate([br, bz, bn])                       # [3H]
    # exact split-K layout: rows = xhi*Whi | 1*bhi | xlo*Whi | 1*blo | xhi*Wlo
    w_hi = _round11(wcat)
    w_lo = wcat - w_hi
    b_hi = _round11(bcat)
    b_lo = bcat - b_hi
    wi = np.concatenate([
        w_hi.T, b_hi[None, :], w_hi.T, b_lo[None, :], w_lo.T], axis=0)

    wms = f32("wms_w")                                        # [2, H, H]
    wsc = f32("w_w").T.reshape(HT, P, 1).transpose(1, 0, 2)   # [P, HT, 1]
    wout = lhsT_h(np.concatenate([f32("mu_w"), f32("lv_w")], axis=0))

    biases = np.zeros((P, 9), dtype=np.float32)
    wms_b = f32("wms_b")                                      # [2, H]
    for k in range(2):
        for mt in range(HT):
            biases[:, 2 * k + mt] = wms_b[k, mt * P:(mt + 1) * P]
    wa_b = f32("wa_b")
    for mt in range(HT):
        biases[:, 4 + mt] = wa_b[mt * P:(mt + 1) * P]
    biases[:, 6] = f32("mu_b")
    biases[:, 7] = f32("lv_b")
    biases[:, 8] = float(np.asarray(inputs["w_b"]).reshape(-1)[0])

    packed = {
        "wi": _to_dt(wi, mode),
        "whr": _to_dt(lhsT_h(f32("whr_w")), mode),
        "whz": _to_dt(lhsT_h(f32("whz_w")), mode),
        "whn": _to_dt(lhsT_h(f32("whn_w")), mode),
        "wa": _to_dt(lhsT_h(f32("wa_w")), mode),
        "wms0": _to_dt(lhsT_h(wms[0]), mode),
        "wms1": _to_dt(lhsT_h(wms[1]), mode),
        "wsc": _to_dt(wsc, mode),
        "wout": _to_dt(wout, mode),
        "ones1": _to_dt(np.ones((1, P)), mode),
        "biases": biases,
    }
    use_bias = any(
        float(np.abs(np.asarray(inputs[k])).max()) != 0.0
        for k in ("wms_b", "wa_b", "w_b", "mu_b", "lv_b")
    )
    return packed, use_bias


def _pack_percore(targets, mask, mode, depth=DEPTH, ncol=NCOL,
                  ncores=NCORES):
    order = _post_order(depth)
    perm = np.array([_gid(d, j) for (d, j) in order])
    nodes = len(order)
    bsz = targets.shape[1]

    tg = np.asarray(targets, dtype=np.float32)[:, :, 0, :]    # [nodes, B, I]
    xall = tg.transpose(2, 0, 1)[:, perm, :]                  # [I, nodes, B]
    x_hi = _round11(xall)
    x_lo = xall - x_hi
    ones = np.ones((1, nodes, bsz), np.float32)
    xaug = np.concatenate([x_hi, ones, x_lo, ones, x_hi], axis=0)  # [KSP,...]
    mpost = np.asarray(mask, dtype=np.float32)[perm]          # [nodes, B]

    xaug = _to_dt(xaug, mode)
    mpost = _to_dt(mpost, mode)

    per_core = []
    for c in range(ncores):
        cols = slice(c * ncol, (c + 1) * ncol)
        xc = np.ascontiguousarray(xaug[:, :, cols])
        mc = np.ascontiguousarray(
            np.broadcast_to(mpost[None, :, cols], (P, nodes, ncol)))
        per_core.append({"xT": xc, "maskb": mc})
    return per_core


def kernel(**inputs):
    import sys
    try:
        import concourse.bass  # noqa: F401
    except ImportError:
        sys.path.insert(0, "/opt/trn_rl_repo")

    try:
        import antenv.axon_hooks  # noqa: F401
    except ImportError:
        # absent in trimmed containers; run_bass_kernel_spmd imports it
        # unconditionally when BASS_TRACE is set — stub the no-hook path
        import types
        _m = types.ModuleType("antenv.axon_hooks")
        _m.get_axon_ntff_profile_hook = lambda: None
        sys.modules["antenv.axon_hooks"] = _m

    from concourse import bass_utils

    mode = "f32"
    packed, use_bias = _pack_weights(inputs, mode)
    use_mask = bool(np.any(np.asarray(inputs["mask"]) != 1.0))
    nc = _build_module(use_bias=use_bias, mode=mode, use_mask=use_mask)
    per_core = _pack_percore(inputs["targets"], inputs["mask"], mode)

    in_maps = [{**pc, **packed} for pc in per_core]
    res = bass_utils.run_bass_kernel_spmd(
        nc, in_maps, core_ids=list(range(NCORES)))

    mu = np.empty((B, 1, O), dtype=np.float32)
    lv = np.empty((B, 1, O), dtype=np.float32)
    for c in range(NCORES):
        out = res.results[c]["out"]                          # [2, P, ncol]
        cols = slice(c * NCOL, (c + 1) * NCOL)
        mu[cols, 0, :] = out[0].T
        lv[cols, 0, :] = out[1].T
    return mu, lv

